# revision 29
# baseline (speedup 1.0000x reference)
"""GPT2-style fused attention (DecisionTransformer) on 8 trn2 NeuronCores.

Sharding: tensor-parallel over heads (16 heads -> 2 per core). Each core:
  - computes QKV^T projection for its 2 heads (both batches)
  - attention per (batch, head) pair in dual layout:
      * natural S[q,k] for the attn_weights output (softmax via exp + causal
        additive mask on the diagonal chunk, normalize with accum sums)
      * transposed S^T[k,q] for the AV path (ones-row appended to V gives
        column sums; K=1 PE-broadcast of 1/sum normalizes O^T)
  - c_proj partial product (contraction over this core's head dims)
Host sums the 8 c_proj partials, adds biases, scatters attn_weights.

Causal structure is exploited: fully-masked tiles are skipped (their
attn_weights stay as the zero-initialized output buffer contents).
If the attention_mask input is NOT the standard causal(-1e9) mask, a
generic fallback path loads the mask explicitly.
"""

import numpy as np

B, S, D, H = 2, 2048, 1024, 16
HD = D // H          # 64
HPC = 2              # heads per core
NC = 8               # cores
SCALE = 1.0 / np.sqrt(HD)
NEG = -1e9

_CACHE = {}


def _build(causal: bool):
    import concourse.bass as bass
    import concourse.tile as tile
    from concourse import bacc, mybir
    from concourse.masks import make_identity

    f32 = mybir.dt.float32
    f32r = mybir.dt.float32r
    AF = mybir.ActivationFunctionType
    ADD = mybir.AluOpType.add

    nc = bacc.Bacc("TRN2", target_bir_lowering=False, debug=False, num_devices=NC)

    hs = nc.dram_tensor("hs", [B * S, D], f32, kind="ExternalInput").ap()
    wqkv = nc.dram_tensor("wqkv", [D, 3 * HPC * HD], f32, kind="ExternalInput").ap()
    bqkv = nc.dram_tensor("bqkv", [3, HPC * HD], f32, kind="ExternalInput").ap()
    wp = nc.dram_tensor("wp", [HPC * HD, D], f32, kind="ExternalInput").ap()
    attnw = nc.dram_tensor("attnw", [B, HPC, S, S], f32, kind="ExternalOutput").ap()
    outp = nc.dram_tensor("outp", [B * S, D], f32, kind="ExternalOutput").ap()
    if not causal:
        maskin = nc.dram_tensor("maskin", [B, S, S], f32, kind="ExternalInput").ap()

    NQT = S // 128           # 16 q tiles per pair
    NKC = S // 512           # 4  k chunks of 512
    NIC = D // 128           # 8  in-dim chunks

    with tile.TileContext(nc) as tc:
        with tc.tile_pool(name="sb", bufs=1) as sb, \
             tc.tile_pool(name="ps", bufs=1, space="PSUM") as ps:

            # ---- constants ----
            ident = sb.tile([128, 128], f32, tag="ident")
            make_identity(nc, ident[:])
            ones1 = sb.tile([1, 64], f32, tag="ones1")
            nc.vector.memset(ones1[:], 1.0)
            ones1r = sb.tile([1, 64], f32r, tag="ones1r")
            nc.vector.tensor_copy(ones1r[:], ones1[:])
            ones16 = sb.tile([128, 16], f32, tag="ones16")
            nc.vector.memset(ones16[:], 1.0)
            # additive causal masks for the diagonal chunks, bf16
            # A_v (path A, [q,k] layout): keep jj <= i + 128v
            # Bt_w (path B, [k,q] layout): keep jj >= i + 128w
            bf16 = mybir.dt.bfloat16
            maskA = []
            if causal:
                for v in range(4):
                    m = sb.tile([128, 512], bf16, tag=f"maskA{v}")
                    nc.gpsimd.memset(m[:], 0.0)
                    nc.gpsimd.affine_select(
                        out=m[:], in_=m[:], compare_op=mybir.AluOpType.is_ge,
                        fill=NEG, base=128 * v, pattern=[[-1, 512]],
                        channel_multiplier=1)
                    maskA.append(m)

            # ---- weights ----
            # wqkv [1024, 384] -> [128, 8(ic), 384] (partition = row within chunk)
            # DMA-cast f32 -> f32r during the (SWDGE) load.
            wq_r = sb.tile([128, NIC, 3 * HPC * HD], f32r, tag="wqr")
            nc.gpsimd.dma_start(
                wq_r[:], wqkv.rearrange("(i p) c -> p i c", p=128))
            bq_sb = sb.tile([128, 3], f32, tag="bq")
            nc.sync.dma_start(bq_sb[:], bqkv.rearrange("m p -> p m"))
            wp_r = sb.tile([64, HPC, D], f32r, tag="wpr")
            nc.gpsimd.dma_start(wp_r[:], wp.rearrange("(j p) c -> p j c", p=64))

            # ---- per batch: QKV^T projection, attention, c_proj ----
            # QKV^T layout: partition p = (head j = p//64, hd = p%64), free = s
            for b in range(B):
                # phase 0: X^T build + QKV^T, in two s-halves
                qkv_r = {}   # m -> tile [128, 2048] (0=Q^T 1=K^T f32r, 2=V^T f32)
                for m in range(3):
                    dt_out = f32 if m == 2 else f32r
                    qkv_r[m] = sb.tile([128, S], dt_out, tag=f"qkv{m}",
                                       bufs=2, name=f"qkv_{b}_{m}")
                for sh in range(2):
                    s0 = 1024 * sh
                    xt = sb.tile([128, NIC, 1024], f32r, tag="xt")
                    for rb in range(8):
                        xld = sb.tile([128, D], f32, tag="xld", bufs=3)
                        nc.sync.dma_start(
                            xld[:], hs[b * S + s0 + 128 * rb:
                                       b * S + s0 + 128 * (rb + 1), :])
                        for ic4 in range(2):
                            tp4 = ps.tile([128, 512], f32, tag="c1", bufs=6)
                            for ii in range(4):
                                ic = 4 * ic4 + ii
                                nc.tensor.transpose(
                                    tp4[:, 128 * ii:128 * (ii + 1)],
                                    xld[:, 128 * ic:128 * (ic + 1)], ident[:])
                            nc.vector.tensor_copy(
                                xt[:, 4 * ic4:4 * ic4 + 4,
                                   128 * rb:128 * (rb + 1)],
                                tp4[:].rearrange("p (i c) -> p i c", i=4))
                    for m in range(3):
                        for rc in range(2):
                            acc = ps.tile([128, 512], f32, tag="c1", bufs=6)
                            for ic in range(NIC):
                                nc.tensor.matmul(
                                    acc[:],
                                    wq_r[:, ic, 128 * m:128 * (m + 1)],
                                    xt[:, ic, 512 * rc:512 * (rc + 1)],
                                    start=(ic == 0), stop=(ic == NIC - 1))
                            nc.vector.tensor_scalar(
                                out=qkv_r[m][:, s0 + 512 * rc:s0 + 512 * (rc + 1)],
                                in0=acc[:], scalar1=bq_sb[:, m:m + 1],
                                scalar2=None, op0=ADD)

                # ---- attention, both heads interleaved ----
                # j=0 ops sit at partitions 0-63, j=1 at 64-127: adjacent
                # matmuls land in different PE row-groups and overlap.
                qT = qkv_r[0]
                kT = qkv_r[1]
                vT = qkv_r[2]
                ot_sb = {}       # j -> normalized O^T [64, 2048] f32r

                # path A: natural scores -> attn_weights
                for qg in range(NQT // 2):
                    sumG = {j: sb.tile([128, 2], f32, tag=f"sumG{j}", bufs=2,
                                       name=f"sumG_{b}_{j}_{qg}")
                            for j in range(HPC)}
                    probs4 = []
                    for g in range(2):
                        qt = 2 * qg + g
                        q0 = 128 * qt
                        nkc = qt // 4 + 1
                        probs = {j: sb.tile([128, S], f32, tag=f"probs{j}",
                                            bufs=2, name=f"probs_{b}_{j}_{qt}")
                                 for j in range(HPC)}
                        probs4.append((qt, nkc, probs))
                        sumP = {j: sb.tile([128, 4], f32, tag=f"sumP{j}", bufs=3,
                                           name=f"sumP_{b}_{j}_{qt}")
                                for j in range(HPC)}
                        for kc in range(nkc):
                            psA = {}
                            for j in range(HPC):
                                hd0 = 64 * j
                                pA = ps.tile([128, 512], f32, tag="c1", bufs=6,
                                             name=f"psA_{j}")
                                psA[j] = pA
                                nc.tensor.matmul(
                                    pA[:],
                                    qT[hd0:hd0 + 64, q0:q0 + 128],
                                    kT[hd0:hd0 + 64, 512 * kc:512 * (kc + 1)],
                                    start=True, stop=True)
                            for j in range(HPC):
                                if causal:
                                    if kc == nkc - 1:
                                        nc.vector.tensor_tensor(
                                            out=psA[j][:], in0=psA[j][:],
                                            in1=maskA[qt % 4][:], op=ADD)
                                else:
                                    mtile = sb.tile([128, 512], f32, tag="mld",
                                                    bufs=3, name=f"mld_{j}")
                                    nc.sync.dma_start(
                                        mtile[:],
                                        maskin[b, q0:q0 + 128,
                                               512 * kc:512 * (kc + 1)])
                                    nc.vector.tensor_tensor(
                                        out=psA[j][:], in0=psA[j][:],
                                        in1=mtile[:], op=ADD)
                                nc.scalar.activation(
                                    probs[j][:, 512 * kc:512 * (kc + 1)],
                                    psA[j][:], AF.Exp, bias=0.0,
                                    scale=float(SCALE),
                                    accum_out=sumP[j][:, kc:kc + 1])
                        for j in range(HPC):
                            nc.vector.reduce_sum(
                                out=sumG[j][:, g:g + 1], in_=sumP[j][:, 0:nkc],
                                axis=mybir.AxisListType.X)
                    invG = {}
                    for j in range(HPC):
                        iG = sb.tile([128, 2], f32, tag=f"invG{j}", bufs=2,
                                     name=f"invG_{j}")
                        invG[j] = iG
                        nc.vector.reciprocal(iG[:], sumG[j][:])
                    for g, (qt, nkc, probs) in enumerate(probs4):
                        q0 = 128 * qt
                        for j in range(HPC):
                            nc.vector.tensor_scalar(
                                out=probs[j][:, :512 * nkc],
                                in0=probs[j][:, :512 * nkc],
                                scalar1=invG[j][:, g:g + 1], scalar2=None,
                                op0=mybir.AluOpType.mult)
                            nc.sync.dma_start(
                                attnw[b, j, q0:q0 + 128, 0:512 * nkc],
                                probs[j][:, :512 * nkc])

                # path B prep: V chunks (transposed) with ones column
                vones = {}
                for j in range(HPC):
                    hd0 = 64 * j
                    vo = sb.tile([128, NQT, 65], f32r, tag=f"vones{j}",
                                 name=f"vones_{b}_{j}")
                    vones[j] = vo
                    nc.vector.tensor_copy(vo[:, :, 64], ones16[:])
                    for kt4 in range(NQT // 4):
                        tp4 = ps.tile([128, 512], f32, tag="c1", bufs=6,
                                      name=f"tp4_{j}")
                        for ii in range(4):
                            kt = 4 * kt4 + ii
                            nc.tensor.transpose(
                                tp4[:, 128 * ii:128 * ii + 64],
                                vT[hd0:hd0 + 64, 128 * kt:128 * (kt + 1)],
                                ident[hd0:hd0 + 64, hd0:hd0 + 64])
                        nc.vector.tensor_copy(
                            vo[:, 4 * kt4:4 * kt4 + 4, 0:64],
                            tp4[:].rearrange("p (i c) -> p i c", i=4)[:, :, 0:64])

                # path B: S^T -> exp -> AV (+sums) -> normalized O^T
                for j in range(HPC):
                    ot_sb[j] = sb.tile([64, S], f32r, tag=f"ot{j}",
                                       name=f"ot_{b}_{j}")
                PIPE = 2
                for qc in range(NKC):
                    av = {j: ps.tile([65, 512], f32, tag=f"av{j}", bufs=1,
                                     name=f"av_{j}")
                          for j in range(HPC)}
                    nkt = 4 * (qc + 1)
                    ests = {}
                    for kti in range(nkt + PIPE):
                        if kti < nkt:
                            kt = kti
                            st = {}
                            for j in range(HPC):
                                hd0 = 64 * j
                                s_t = ps.tile([128, 512], f32, tag="c1", bufs=6,
                                              name=f"st_{j}")
                                st[j] = s_t
                                nc.tensor.matmul(
                                    s_t[:],
                                    kT[hd0:hd0 + 64, 128 * kt:128 * (kt + 1)],
                                    qT[hd0:hd0 + 64, 512 * qc:512 * (qc + 1)],
                                    start=True, stop=True)
                            for j in range(HPC):
                                if not causal:
                                    mtile = sb.tile([128, 512], f32, tag="mld",
                                                    bufs=3, name=f"mldB_{j}")
                                    nc.sync.dma_start(
                                        mtile[:],
                                        maskin[b, 512 * qc:512 * (qc + 1),
                                               128 * kt:128 * (kt + 1)].rearrange(
                                                   "q k -> k q"))
                                    nc.vector.tensor_tensor(
                                        out=st[j][:], in0=st[j][:], in1=mtile[:],
                                        op=ADD)
                                est = sb.tile([128, 512], f32r, tag=f"est{j}",
                                              bufs=PIPE + 1, name=f"est_{j}")
                                nc.scalar.activation(
                                    est[:], st[j][:], AF.Exp, bias=0.0,
                                    scale=float(SCALE))
                                if causal and kt >= 4 * qc:
                                    w = kt - 4 * qc
                                    nc.gpsimd.affine_select(
                                        out=est[:], in_=est[:],
                                        compare_op=mybir.AluOpType.is_ge,
                                        fill=0.0, base=-128 * w,
                                        pattern=[[1, 512]],
                                        channel_multiplier=-1)
                                ests[(kt, j)] = est
                        if kti >= PIPE:
                            kt = kti - PIPE
                            for j in range(HPC):
                                nc.tensor.matmul(
                                    av[j][:], vones[j][:, kt, :],
                                    ests.pop((kt, j))[:],
                                    start=(kt == 0), stop=(kt == nkt - 1))
                    # normalize O^T chunk: x (1/sums) broadcast via K=1 matmul
                    for j in range(HPC):
                        invrow = sb.tile([1, 512], f32r, tag=f"invrow{j}",
                                         bufs=2, name=f"invrow_{j}")
                        with nc.allow_low_precision(reason="f32r broadcast"):
                            nc.vector.reciprocal(invrow[:], av[j][64:65, :])
                        bc = ps.tile([64, 512], f32, tag="c1", bufs=6,
                                     name=f"bc_{j}")
                        nc.tensor.matmul(bc[:], ones1r[:],
                                         invrow[:], start=True, stop=True)
                        bcs = sb.tile([64, 512], f32, tag=f"bcs{j}", bufs=2,
                                      name=f"bcs_{j}")
                        nc.vector.tensor_copy(bcs[:], bc[:])
                        nc.vector.tensor_tensor(
                            out=ot_sb[j][:, 512 * qc:512 * (qc + 1)],
                            in0=av[j][0:64, :], in1=bcs[:],
                            op=mybir.AluOpType.mult)

                # ---- c_proj partial for this batch ----
                for qt in range(NQT):
                    q0 = 128 * qt
                    for ncc in range(2):
                        cp = ps.tile([128, 512], f32, tag="c1", bufs=6)
                        for j in range(HPC):
                            nc.tensor.matmul(
                                cp[:],
                                ot_sb[j][:, q0:q0 + 128],
                                wp_r[:, j, 512 * ncc:512 * (ncc + 1)],
                                start=(j == 0), stop=(j == HPC - 1))
                        cps = sb.tile([128, 512], f32, tag="cps", bufs=3)
                        nc.any.tensor_copy(cps[:], cp[:])
                        nc.sync.dma_start(
                            outp[b * S + q0:b * S + q0 + 128,
                                 512 * ncc:512 * (ncc + 1)],
                            cps[:])

    nc.compile()
    return nc


def _get_nc(causal: bool):
    if causal not in _CACHE:
        _CACHE[causal] = _build(causal)
    return _CACHE[causal]


def _is_causal_mask(attention_mask: np.ndarray) -> bool:
    if attention_mask.shape != (B, 1, S, S):
        return False
    m0 = attention_mask[0, 0]
    iidx = np.arange(S)
    low = iidx[:, None] >= iidx[None, :]
    if not np.all(m0[low] == 0.0):
        return False
    if not np.all(m0[~low] == np.float32(NEG)):
        return False
    return bool(np.all(attention_mask == m0[None, None]))


def kernel(hidden_states, attention_mask, c_attn_w, c_attn_b, c_proj_w, c_proj_b):
    from concourse.bass_utils import run_bass_kernel_spmd

    hidden_states = np.ascontiguousarray(np.asarray(hidden_states, dtype=np.float32))
    attention_mask = np.asarray(attention_mask, dtype=np.float32)
    c_attn_w = np.asarray(c_attn_w, dtype=np.float32)
    c_attn_b = np.asarray(c_attn_b, dtype=np.float32)
    c_proj_w = np.asarray(c_proj_w, dtype=np.float32)
    c_proj_b = np.asarray(c_proj_b, dtype=np.float32)

    causal = _is_causal_mask(attention_mask)
    nc = _get_nc(causal)

    hs = hidden_states.reshape(B * S, D)
    in_maps = []
    for c in range(NC):
        h0, h1 = HPC * c, HPC * c + 1
        cols = np.r_[h0 * HD:(h0 + 1) * HD, h1 * HD:(h1 + 1) * HD]
        wqkv = np.concatenate(
            [c_attn_w[:, cols], c_attn_w[:, D + cols], c_attn_w[:, 2 * D + cols]],
            axis=1)
        bqkv = np.stack(
            [c_attn_b[cols], c_attn_b[D + cols], c_attn_b[2 * D + cols]], axis=0)
        wp = c_proj_w[cols, :]
        im = {"hs": hs, "wqkv": np.ascontiguousarray(wqkv),
              "bqkv": np.ascontiguousarray(bqkv), "wp": np.ascontiguousarray(wp)}
        if not causal:
            im["maskin"] = np.ascontiguousarray(
                np.broadcast_to(attention_mask[:, 0], (B, S, S)))
        in_maps.append(im)

    res = run_bass_kernel_spmd(nc, in_maps, list(range(NC)))

    attn_output = np.zeros((B * S, D), dtype=np.float32)
    attn_weights = np.empty((B, H, S, S), dtype=np.float32)
    for c in range(NC):
        r = res.results[c]
        attn_output += r["outp"]
        attn_weights[:, HPC * c:HPC * (c + 1)] = r["attnw"]
    attn_output += c_proj_b[None, :]
    return attn_output.reshape(B, S, D), attn_weights


# revision 30
# speedup vs baseline: 1.0741x; 1.0741x over previous
"""GPT2-style fused attention (DecisionTransformer) on 8 trn2 NeuronCores.

Sharding: tensor-parallel over heads (16 heads -> 2 per core). Each core:
  - computes QKV^T projection for its 2 heads (both batches)
  - attention per (batch, head) pair in dual layout:
      * natural S[q,k] for the attn_weights output (softmax via exp + causal
        additive mask on the diagonal chunk, normalize with accum sums)
      * transposed S^T[k,q] for the AV path (ones-row appended to V gives
        column sums; K=1 PE-broadcast of 1/sum normalizes O^T)
  - c_proj partial product (contraction over this core's head dims)
Host sums the 8 c_proj partials, adds biases, scatters attn_weights.

Causal structure is exploited: fully-masked tiles are skipped (their
attn_weights stay as the zero-initialized output buffer contents).
If the attention_mask input is NOT the standard causal(-1e9) mask, a
generic fallback path loads the mask explicitly.
"""

import numpy as np

B, S, D, H = 2, 2048, 1024, 16
HD = D // H          # 64
HPC = 2              # heads per core
NC = 8               # cores
SCALE = 1.0 / np.sqrt(HD)
NEG = -1e9

_CACHE = {}


def _build(causal: bool):
    import concourse.bass as bass
    import concourse.tile as tile
    from concourse import bacc, mybir
    from concourse.masks import make_identity

    f32 = mybir.dt.float32
    f32r = mybir.dt.float32r
    fp16 = mybir.dt.float16
    AF = mybir.ActivationFunctionType
    ADD = mybir.AluOpType.add

    nc = bacc.Bacc("TRN2", target_bir_lowering=False, debug=False, num_devices=NC)

    hs = nc.dram_tensor("hs", [B * S, D], f32, kind="ExternalInput").ap()
    wqkv = nc.dram_tensor("wqkv", [D, 3 * HPC * HD], f32, kind="ExternalInput").ap()
    bqkv = nc.dram_tensor("bqkv", [3, HPC * HD], f32, kind="ExternalInput").ap()
    wp = nc.dram_tensor("wp", [HPC * HD, D], f32, kind="ExternalInput").ap()
    attnw = nc.dram_tensor("attnw", [B, HPC, S, S], f32, kind="ExternalOutput").ap()
    outp = nc.dram_tensor("outp", [B * S, D], f32, kind="ExternalOutput").ap()
    if not causal:
        maskin = nc.dram_tensor("maskin", [B, S, S], f32, kind="ExternalInput").ap()

    NQT = S // 128           # 16 q tiles per pair
    NKC = S // 512           # 4  k chunks of 512
    NIC = D // 128           # 8  in-dim chunks

    with tile.TileContext(nc) as tc:
        with tc.tile_pool(name="sb", bufs=1) as sb, \
             tc.tile_pool(name="ps", bufs=1, space="PSUM") as ps:

            # ---- constants ----
            ident = sb.tile([128, 128], f32, tag="ident")
            make_identity(nc, ident[:])
            ones1 = sb.tile([1, 64], f32, tag="ones1")
            nc.vector.memset(ones1[:], 1.0)
            ones1r = sb.tile([1, 64], f32r, tag="ones1r")
            nc.vector.tensor_copy(ones1r[:], ones1[:])
            ones16 = sb.tile([128, 16], f32, tag="ones16")
            nc.vector.memset(ones16[:], 1.0)
            # additive causal masks for the diagonal chunks, bf16
            # A_v (path A, [q,k] layout): keep jj <= i + 128v
            # Bt_w (path B, [k,q] layout): keep jj >= i + 128w
            bf16 = mybir.dt.bfloat16
            maskA = []
            if causal:
                for v in range(4):
                    m = sb.tile([128, 512], bf16, tag=f"maskA{v}")
                    nc.gpsimd.memset(m[:], 0.0)
                    nc.gpsimd.affine_select(
                        out=m[:], in_=m[:], compare_op=mybir.AluOpType.is_ge,
                        fill=NEG, base=128 * v, pattern=[[-1, 512]],
                        channel_multiplier=1)
                    maskA.append(m)

            # ---- weights ----
            # wqkv [1024, 384] -> [128, 8(ic), 384] (partition = row within chunk)
            # DMA-cast f32 -> f32r during the (SWDGE) load.
            wq_r = sb.tile([128, NIC, 3 * HPC * HD], f32r, tag="wqr")
            nc.gpsimd.dma_start(
                wq_r[:], wqkv.rearrange("(i p) c -> p i c", p=128))
            bq_sb = sb.tile([128, 3], f32, tag="bq")
            nc.sync.dma_start(bq_sb[:], bqkv.rearrange("m p -> p m"))
            wp_r = sb.tile([64, HPC, D], fp16, tag="wpr")
            nc.gpsimd.dma_start(wp_r[:], wp.rearrange("(j p) c -> p j c", p=64))

            # ---- per batch: QKV^T projection, attention, c_proj ----
            # QKV^T layout: partition p = (head j = p//64, hd = p%64), free = s
            for b in range(B):
                # phase 0: X^T build + QKV^T, in two s-halves
                qkv_r = {}   # m -> tile [128, 2048] (0=Q^T 1=K^T f32r, 2=V^T f32)
                for m in range(3):
                    dt_out = f32 if m == 2 else f32r
                    qkv_r[m] = sb.tile([128, S], dt_out, tag=f"qkv{m}",
                                       bufs=2, name=f"qkv_{b}_{m}")
                for sh in range(2):
                    s0 = 1024 * sh
                    xt = sb.tile([128, NIC, 1024], f32r, tag="xt")
                    for rb in range(8):
                        xld = sb.tile([128, D], f32, tag="xld", bufs=3)
                        nc.sync.dma_start(
                            xld[:], hs[b * S + s0 + 128 * rb:
                                       b * S + s0 + 128 * (rb + 1), :])
                        for ic4 in range(2):
                            tp4 = ps.tile([128, 512], f32, tag="c1", bufs=6)
                            for ii in range(4):
                                ic = 4 * ic4 + ii
                                nc.tensor.transpose(
                                    tp4[:, 128 * ii:128 * (ii + 1)],
                                    xld[:, 128 * ic:128 * (ic + 1)], ident[:])
                            nc.vector.tensor_copy(
                                xt[:, 4 * ic4:4 * ic4 + 4,
                                   128 * rb:128 * (rb + 1)],
                                tp4[:].rearrange("p (i c) -> p i c", i=4))
                    for m in range(3):
                        for rc in range(2):
                            acc = ps.tile([128, 512], f32, tag="c1", bufs=6)
                            for ic in range(NIC):
                                nc.tensor.matmul(
                                    acc[:],
                                    wq_r[:, ic, 128 * m:128 * (m + 1)],
                                    xt[:, ic, 512 * rc:512 * (rc + 1)],
                                    start=(ic == 0), stop=(ic == NIC - 1))
                            nc.vector.tensor_scalar(
                                out=qkv_r[m][:, s0 + 512 * rc:s0 + 512 * (rc + 1)],
                                in0=acc[:], scalar1=bq_sb[:, m:m + 1],
                                scalar2=None, op0=ADD)

                # ---- attention, both heads interleaved ----
                # j=0 ops sit at partitions 0-63, j=1 at 64-127: adjacent
                # matmuls land in different PE row-groups and overlap.
                qT = qkv_r[0]
                kT = qkv_r[1]
                vT = qkv_r[2]
                ot_sb = {}       # j -> normalized O^T [64, 2048] f32r

                # path A: natural scores -> attn_weights
                for qg in range(NQT // 2):
                    sumG = {j: sb.tile([128, 2], f32, tag=f"sumG{j}", bufs=2,
                                       name=f"sumG_{b}_{j}_{qg}")
                            for j in range(HPC)}
                    probs4 = []
                    for g in range(2):
                        qt = 2 * qg + g
                        q0 = 128 * qt
                        nkc = qt // 4 + 1
                        probs = {j: sb.tile([128, S], f32, tag=f"probs{j}",
                                            bufs=3, name=f"probs_{b}_{j}_{qt}")
                                 for j in range(HPC)}
                        probs4.append((qt, nkc, probs))
                        sumP = {j: sb.tile([128, 4], f32, tag=f"sumP{j}", bufs=3,
                                           name=f"sumP_{b}_{j}_{qt}")
                                for j in range(HPC)}
                        for kc in range(nkc):
                            psA = {}
                            for j in range(HPC):
                                hd0 = 64 * j
                                pA = ps.tile([128, 512], f32, tag="c1", bufs=6,
                                             name=f"psA_{j}")
                                psA[j] = pA
                                nc.tensor.matmul(
                                    pA[:],
                                    qT[hd0:hd0 + 64, q0:q0 + 128],
                                    kT[hd0:hd0 + 64, 512 * kc:512 * (kc + 1)],
                                    start=True, stop=True)
                            for j in range(HPC):
                                if causal:
                                    if kc == nkc - 1:
                                        nc.vector.tensor_tensor(
                                            out=psA[j][:], in0=psA[j][:],
                                            in1=maskA[qt % 4][:], op=ADD)
                                else:
                                    mtile = sb.tile([128, 512], f32, tag="mld",
                                                    bufs=3, name=f"mld_{j}")
                                    nc.sync.dma_start(
                                        mtile[:],
                                        maskin[b, q0:q0 + 128,
                                               512 * kc:512 * (kc + 1)])
                                    nc.vector.tensor_tensor(
                                        out=psA[j][:], in0=psA[j][:],
                                        in1=mtile[:], op=ADD)
                                nc.scalar.activation(
                                    probs[j][:, 512 * kc:512 * (kc + 1)],
                                    psA[j][:], AF.Exp, bias=0.0,
                                    scale=float(SCALE),
                                    accum_out=sumP[j][:, kc:kc + 1])
                        for j in range(HPC):
                            nc.vector.reduce_sum(
                                out=sumG[j][:, g:g + 1], in_=sumP[j][:, 0:nkc],
                                axis=mybir.AxisListType.X)
                    invG = {}
                    for j in range(HPC):
                        iG = sb.tile([128, 2], f32, tag=f"invG{j}", bufs=2,
                                     name=f"invG_{j}")
                        invG[j] = iG
                        nc.vector.reciprocal(iG[:], sumG[j][:])
                    for g, (qt, nkc, probs) in enumerate(probs4):
                        q0 = 128 * qt
                        for j in range(HPC):
                            nc.vector.tensor_scalar(
                                out=probs[j][:, :512 * nkc],
                                in0=probs[j][:, :512 * nkc],
                                scalar1=invG[j][:, g:g + 1], scalar2=None,
                                op0=mybir.AluOpType.mult)
                            nc.sync.dma_start(
                                attnw[b, j, q0:q0 + 128, 0:512 * nkc],
                                probs[j][:, :512 * nkc])

                # path B prep: V chunks (transposed) with ones column
                vones = {}
                for j in range(HPC):
                    hd0 = 64 * j
                    vo = sb.tile([128, NQT, 65], fp16, tag=f"vones{j}",
                                 name=f"vones_{b}_{j}")
                    vones[j] = vo
                    nc.vector.tensor_copy(vo[:, :, 64], ones16[:])
                    for kt4 in range(NQT // 4):
                        tp4 = ps.tile([128, 512], f32, tag="c1", bufs=6,
                                      name=f"tp4_{j}")
                        for ii in range(4):
                            kt = 4 * kt4 + ii
                            nc.tensor.transpose(
                                tp4[:, 128 * ii:128 * ii + 64],
                                vT[hd0:hd0 + 64, 128 * kt:128 * (kt + 1)],
                                ident[hd0:hd0 + 64, hd0:hd0 + 64])
                        nc.vector.tensor_copy(
                            vo[:, 4 * kt4:4 * kt4 + 4, 0:64],
                            tp4[:].rearrange("p (i c) -> p i c", i=4)[:, :, 0:64])

                # path B: S^T -> exp -> AV (+sums) -> normalized O^T
                for j in range(HPC):
                    ot_sb[j] = sb.tile([64, S], fp16, tag=f"ot{j}",
                                       name=f"ot_{b}_{j}")
                PIPE = 4
                for qc in range(NKC):
                    av = {j: ps.tile([65, 512], f32, tag=f"av{j}", bufs=1,
                                     name=f"av_{j}")
                          for j in range(HPC)}
                    nkt = 4 * (qc + 1)
                    ests = {}
                    for kti in range(nkt + PIPE):
                        if kti < nkt:
                            kt = kti
                            st = {}
                            for j in range(HPC):
                                hd0 = 64 * j
                                s_t = ps.tile([128, 512], f32, tag="c1", bufs=6,
                                              name=f"st_{j}")
                                st[j] = s_t
                                nc.tensor.matmul(
                                    s_t[:],
                                    kT[hd0:hd0 + 64, 128 * kt:128 * (kt + 1)],
                                    qT[hd0:hd0 + 64, 512 * qc:512 * (qc + 1)],
                                    start=True, stop=True)
                            for j in range(HPC):
                                if not causal:
                                    mtile = sb.tile([128, 512], f32, tag="mld",
                                                    bufs=3, name=f"mldB_{j}")
                                    nc.sync.dma_start(
                                        mtile[:],
                                        maskin[b, 512 * qc:512 * (qc + 1),
                                               128 * kt:128 * (kt + 1)].rearrange(
                                                   "q k -> k q"))
                                    nc.vector.tensor_tensor(
                                        out=st[j][:], in0=st[j][:], in1=mtile[:],
                                        op=ADD)
                                est = sb.tile([128, 512], fp16, tag=f"est{j}",
                                              bufs=PIPE + 1, name=f"est_{j}")
                                nc.scalar.activation(
                                    est[:], st[j][:], AF.Exp, bias=0.0,
                                    scale=float(SCALE))
                                if causal and kt >= 4 * qc:
                                    w = kt - 4 * qc
                                    nc.gpsimd.affine_select(
                                        out=est[:], in_=est[:],
                                        compare_op=mybir.AluOpType.is_ge,
                                        fill=0.0, base=-128 * w,
                                        pattern=[[1, 512]],
                                        channel_multiplier=-1)
                                ests[(kt, j)] = est
                        if kti >= PIPE:
                            kt = kti - PIPE
                            for j in range(HPC):
                                nc.tensor.matmul(
                                    av[j][:], vones[j][:, kt, :],
                                    ests.pop((kt, j))[:],
                                    start=(kt == 0), stop=(kt == nkt - 1))
                    # normalize O^T chunk: x (1/sums) broadcast via K=1 matmul
                    for j in range(HPC):
                        invrow = sb.tile([1, 512], f32r, tag=f"invrow{j}",
                                         bufs=2, name=f"invrow_{j}")
                        with nc.allow_low_precision(reason="f32r broadcast"):
                            nc.vector.reciprocal(invrow[:], av[j][64:65, :])
                        bc = ps.tile([64, 512], f32, tag="c1", bufs=6,
                                     name=f"bc_{j}")
                        nc.tensor.matmul(bc[:], ones1r[:],
                                         invrow[:], start=True, stop=True)
                        bcs = sb.tile([64, 512], f32, tag=f"bcs{j}", bufs=2,
                                      name=f"bcs_{j}")
                        nc.vector.tensor_copy(bcs[:], bc[:])
                        nc.vector.tensor_tensor(
                            out=ot_sb[j][:, 512 * qc:512 * (qc + 1)],
                            in0=av[j][0:64, :], in1=bcs[:],
                            op=mybir.AluOpType.mult)

                # ---- c_proj partial for this batch ----
                for qt in range(NQT):
                    q0 = 128 * qt
                    for ncc in range(2):
                        cp = ps.tile([128, 512], f32, tag="c1", bufs=6)
                        for j in range(HPC):
                            nc.tensor.matmul(
                                cp[:],
                                ot_sb[j][:, q0:q0 + 128],
                                wp_r[:, j, 512 * ncc:512 * (ncc + 1)],
                                start=(j == 0), stop=(j == HPC - 1))
                        cps = sb.tile([128, 512], f32, tag="cps", bufs=3)
                        nc.any.tensor_copy(cps[:], cp[:])
                        nc.sync.dma_start(
                            outp[b * S + q0:b * S + q0 + 128,
                                 512 * ncc:512 * (ncc + 1)],
                            cps[:])

    nc.compile()
    return nc


def _get_nc(causal: bool):
    if causal not in _CACHE:
        _CACHE[causal] = _build(causal)
    return _CACHE[causal]


def _is_causal_mask(attention_mask: np.ndarray) -> bool:
    if attention_mask.shape != (B, 1, S, S):
        return False
    m0 = attention_mask[0, 0]
    iidx = np.arange(S)
    low = iidx[:, None] >= iidx[None, :]
    if not np.all(m0[low] == 0.0):
        return False
    if not np.all(m0[~low] == np.float32(NEG)):
        return False
    return bool(np.all(attention_mask == m0[None, None]))


def kernel(hidden_states, attention_mask, c_attn_w, c_attn_b, c_proj_w, c_proj_b):
    from concourse.bass_utils import run_bass_kernel_spmd

    hidden_states = np.ascontiguousarray(np.asarray(hidden_states, dtype=np.float32))
    attention_mask = np.asarray(attention_mask, dtype=np.float32)
    c_attn_w = np.asarray(c_attn_w, dtype=np.float32)
    c_attn_b = np.asarray(c_attn_b, dtype=np.float32)
    c_proj_w = np.asarray(c_proj_w, dtype=np.float32)
    c_proj_b = np.asarray(c_proj_b, dtype=np.float32)

    causal = _is_causal_mask(attention_mask)
    nc = _get_nc(causal)

    hs = hidden_states.reshape(B * S, D)
    in_maps = []
    for c in range(NC):
        h0, h1 = HPC * c, HPC * c + 1
        cols = np.r_[h0 * HD:(h0 + 1) * HD, h1 * HD:(h1 + 1) * HD]
        wqkv = np.concatenate(
            [c_attn_w[:, cols], c_attn_w[:, D + cols], c_attn_w[:, 2 * D + cols]],
            axis=1)
        bqkv = np.stack(
            [c_attn_b[cols], c_attn_b[D + cols], c_attn_b[2 * D + cols]], axis=0)
        wp = c_proj_w[cols, :]
        im = {"hs": hs, "wqkv": np.ascontiguousarray(wqkv),
              "bqkv": np.ascontiguousarray(bqkv), "wp": np.ascontiguousarray(wp)}
        if not causal:
            im["maskin"] = np.ascontiguousarray(
                np.broadcast_to(attention_mask[:, 0], (B, S, S)))
        in_maps.append(im)

    res = run_bass_kernel_spmd(nc, in_maps, list(range(NC)))

    attn_output = np.zeros((B * S, D), dtype=np.float32)
    attn_weights = np.empty((B, H, S, S), dtype=np.float32)
    for c in range(NC):
        r = res.results[c]
        attn_output += r["outp"]
        attn_weights[:, HPC * c:HPC * (c + 1)] = r["attnw"]
    attn_output += c_proj_b[None, :]
    return attn_output.reshape(B, S, D), attn_weights
